# revision 7
# baseline (speedup 1.0000x reference)
"""Trainium2 Bass kernel for the 8-step complex DMD recurrence — v2.

Math (matching the reference):
  Ag[0]=A[0], Ag[p]=A[8-p] (p>=1), all complex [M,M].
  window w_t (len 8) starts as the real inputs x_0..x_7; each step
    u2_t = sum_p Ag[p] @ w_t[p]   (complex, [B,M])
  then the window slides.  Output = Re([u2_1..u2_8]) as [B, 8, M].

Strategy v2 (m-row sharding + per-step AllGather + 3-mult complex):
  * core c owns OUTPUT rows K_c = [128c, 128c+128) of the M dim and holds
    row slices Ag[p][K_c, :] of every operator.  Each step it computes the
    fully-reduced u2_t[K_c] locally (contraction over all of M) — no
    cross-core reduction needed, and each step's output rows stream
    straight to the per-core out tensor.
  * u2_t must then be broadcast for the next step's contraction: a 1-chip
    AllGather of the [128, 512] bf16 [re|im] shard (measured AG floor
    ~4.6us vs ReduceScatter ~7.3us; AG also avoids the CCE double-read).
  * complex products use the 3-multiplication (Gauss) form with
    host-precomputed stationaries War=Ar, Wd=Ai-Ar, Wn2=-(Ar+Ai):
      S  += Ar  @ (ur+ui)      (also re-part of x terms: S += Ar @ x)
      RE += Wn2 @ ui
      IM += Wd  @ ur           (also im-part of x terms: IM += Wd @ x)
      re = RE + S,  im = IM + S   (two DVE adds at drain time)
    -> 24 matmuls per complex window position instead of 32, and x terms
    need only War/Wd (Ai itself never ships).
  * per step the serial chain is just: close matmuls (newest u2 term) ->
    ACT stage + 2 DVE adds -> DMA [128,512] -> AllGather -> slot DMA +
    us add.  The x-terms and older-u2 terms of each step are emitted
    before its close, keeping the PE busy through the AllGather wait
    (deeper lookahead measured slower: it delays the close on the
    in-order PE queue).
  * step 8 computes only re, accumulated in a single PSUM region.
"""

import numpy as np

B, L, M = 256, 8, 1024
N_CORES = 8
NK = M // 128   # 8 contraction tiles
P_STEPS = 8

IN_NAMES = ("war", "wd", "wn2", "xw")

_CACHE = {}


def _build_program(reps=1, variant="full", pipeline=False):
    import concourse.bacc as bacc
    import concourse.mybir as mybir
    import concourse.tile as tile
    from concourse.bass import ts

    dt = mybir.dt
    f32 = dt.float32
    bf16 = dt.bfloat16
    wdt = bf16   # stationaries + x + slots
    cdt = bf16   # collective payload
    sdt = bf16   # stage / output element type

    nc = bacc.Bacc("TRN2", target_bir_lowering=False, debug=False,
                   num_devices=N_CORES)

    # Per-core inputs, partition-major layouts prepared on the host:
    #   war/wd: [k=128, (p,kt,m)] -> [128, 8*8*128]   row slices, transposed
    #   wn2:    [128, 7*8*128]                        (p>=1 only)
    #   xw:     [128, (q,kt,b)]  -> [128, 8*8*256]    full x, all cores
    war = nc.dram_tensor("war", [128, L * NK * 128], wdt, kind="ExternalInput")
    wd = nc.dram_tensor("wd", [128, L * NK * 128], wdt, kind="ExternalInput")
    wn2 = nc.dram_tensor("wn2", [128, (L - 1) * NK * 128], wdt,
                         kind="ExternalInput")
    xw = nc.dram_tensor("xw", [128, L * NK * 256], wdt, kind="ExternalInput")
    out = nc.dram_tensor("out", [P_STEPS, 128, 256], sdt,
                         kind="ExternalOutput")

    # Collective buffers (HBM): shard in, gathered u2_t out. One set per
    # rep so benchmark replication adds no false cross-rep dependencies.
    cc_in = [[nc.dram_tensor(f"cc_in{r}_{t}", [128, 512], cdt)
              for t in range(1, 8)] for r in range(reps)]
    cc_out = [[nc.dram_tensor(f"cc_out{r}_{t}", [M, 512], cdt)
               for t in range(1, 8)] for r in range(reps)]

    rg = [list(range(N_CORES))]

    with tile.TileContext(nc) as tc:
        with (
            tc.tile_pool(name="a", bufs=1) as apool,
            tc.tile_pool(name="slot", bufs=7) as slpool,
            tc.tile_pool(name="stg", bufs=2) as stpool,
            tc.tile_pool(name="ps", bufs=4, space="PSUM") as pspool,
        ):
            t_war = apool.tile([128, L * NK * 128], wdt, tag="war")
            t_wd = apool.tile([128, L * NK * 128], wdt, tag="wd")
            t_wn2 = apool.tile([128, (L - 1) * NK * 128], wdt, tag="wn2")
            t_xw = apool.tile([128, L * NK * 256], wdt, tag="xw")

            def ar(p, k):
                return t_war[:, ts(p * NK + k, 128)]

            def wd_(p, k):
                return t_wd[:, ts(p * NK + k, 128)]

            def n2(p, k):
                return t_wn2[:, ts((p - 1) * NK + k, 128)]

            def xv(q, k):
                return t_xw[:, ts(q * NK + k, 256)]

            # Head loads: interleave x / war / wd per position so position-0
            # matmuls start after ~1MB instead of 8MB.  wn2 (first consumed
            # by step 2's close, which always uses position 7) queues after
            # the step-1-critical loads, position 7 first.
            for p in range(L):
                nc.sync.dma_start(t_xw[:, ts(p, NK * 256)],
                                  xw[:, ts(p, NK * 256)])
                nc.sync.dma_start(t_war[:, ts(p, NK * 128)],
                                  war[:, ts(p, NK * 128)])
                nc.sync.dma_start(t_wd[:, ts(p, NK * 128)],
                                  wd[:, ts(p, NK * 128)])
            for p in (7, 1, 2, 3, 4, 5, 6):
                nc.sync.dma_start(t_wn2[:, ts(p - 1, NK * 128)],
                                  wn2[:, ts(p - 1, NK * 128)])

            slots = {}  # (rep, j) -> sbuf tile [128, 8*768] per-ktile [ur|ui|us]
            if variant == "nocc":
                dummy = slpool.tile([128, NK * 768], wdt, tag="dummy")
                nc.vector.memset(dummy[:], 0.0)
                slots["dummy"] = dummy

            rep = 0

            def slot_ap(j, k, part):
                lo = k * 768 + part * 256
                return slots[rep, j][:, lo:lo + 256]

            # PSUM region bases within a [128, 1024] (= 2 bank) step tile.
            # PSUM group start/stop is BANK-granular (start zeroes the whole
            # 2KB bank), so S|RE share bank 0 with ONE start, IM is bank 1.
            S_, RE_, IM_ = 0, 256, 512

            pss = {}      # (rep, t) -> psum tile
            started = {}  # (rep, t) -> [bank0, bank1]

            def mm(t, region, lhsT, rhs, stop=False):
                st = started[rep, t]
                bank = 0 if region < 512 else 1
                nc.tensor.matmul(
                    pss[rep, t][:, region:region + 256], lhsT, rhs,
                    start=not st[bank], stop=stop,
                    skip_group_check=True,
                )
                st[bank] = True

            def emit_xA(t):
                """x-only terms of step t — no u2 dependencies at all."""
                pss[rep, t] = pspool.tile([128, 1024], f32, tag="ps",
                                          name=f"ps{rep}_{t}")
                started[rep, t] = [False, False]
                if t < 8:
                    for p in range(0, 9 - t):
                        q = p + t - 1
                        # t=1 has no close phase: its x groups carry the stops
                        cl = t == 1 and p == 8 - t
                        for k in range(NK):
                            st = cl and k == NK - 1
                            mm(t, S_, ar(p, k), xv(q, k), stop=st)
                            mm(t, IM_, wd_(p, k), xv(q, k), stop=st)
                else:
                    for k in range(NK):
                        mm(t, RE_, ar(0, k), xv(7, k))

            def emit_uA(t):
                """older-u2 terms of step t (slots j <= t-2)."""
                for j in range(1, t - 1):
                    pos = 8 - t + j
                    for k in range(NK):
                        if t < 8:
                            mm(t, S_, ar(pos, k), slot_ap(j, k, 2))
                            mm(t, RE_, n2(pos, k), slot_ap(j, k, 1))
                            mm(t, IM_, wd_(pos, k), slot_ap(j, k, 0))
                        else:
                            mm(t, RE_, ar(j, k), slot_ap(j, k, 2))
                            mm(t, RE_, n2(j, k), slot_ap(j, k, 1))

            def emit_close(t):
                """newest term j = t-1, position 7 — waits on slot t-1.

                Region order S -> RE -> IM so S (and then RE) close early:
                the ACT stage-copy of S and the re-combine DVE add overlap
                the remaining RE/IM close matmuls."""
                j = t - 1
                if t < 8:
                    for k in range(NK):
                        mm(t, S_, ar(7, k), slot_ap(j, k, 2),
                           stop=k == NK - 1)
                    for k in range(NK):
                        mm(t, RE_, n2(7, k), slot_ap(j, k, 1),
                           stop=k == NK - 1)
                    for k in range(NK):
                        mm(t, IM_, wd_(7, k), slot_ap(j, k, 0),
                           stop=k == NK - 1)
                else:
                    for k in range(NK):
                        mm(t, RE_, n2(7, k), slot_ap(j, k, 1))
                        mm(t, RE_, ar(7, k), slot_ap(j, k, 2),
                           stop=k == NK - 1)

            def emit_epilogue(t):
                last = t == 8
                ps = pss[rep, t]
                if last:
                    stg = stpool.tile([128, 256], sdt, tag="stg8", name=f"stg8_{rep}")
                    nc.vector.tensor_copy(stg[:], ps[:, RE_:RE_ + 256])
                    nc.sync.dma_start(out[7], stg[:])
                    return
                # DVE tensor_tensor cannot read two PSUM operands (neuronxcc
                # verifier); stage S to SBUF f32 on the ACT engine first.
                stg = stpool.tile([128, 512], sdt, tag="stg", name=f"stg{rep}_{t}")
                s_f32 = stpool.tile([128, 256], f32, tag="sf32",
                                    name=f"sf32_{rep}_{t}")
                nc.scalar.copy(s_f32[:], ps[:, S_:S_ + 256])
                if t == 1:
                    nc.vector.tensor_copy(stg[:, 0:256], ps[:, S_:S_ + 256])
                else:
                    nc.vector.tensor_add(stg[:, 0:256], ps[:, RE_:RE_ + 256],
                                         s_f32[:])
                nc.vector.tensor_add(stg[:, 256:512], ps[:, IM_:IM_ + 256],
                                     s_f32[:])
                # out DMA rides the ACT engine's HWDGE ring so it never
                # queues ahead of the chain-critical cc_in DMA on SP's ring.
                nc.scalar.dma_start(out[t - 1], stg[:, 0:256])
                if variant == "nocc":
                    slots[rep, t] = slots["dummy"]
                    return
                nc.sync.dma_start(cc_in[rep][t - 1][:], stg[:])
                if variant == "full":
                    nc.gpsimd.collective_compute(
                        "AllGather", mybir.AluOpType.bypass,
                        replica_groups=rg,
                        ins=[cc_in[rep][t - 1][:]],
                        outs=[cc_out[rep][t - 1][:]],
                    )
                slot = slpool.tile([128, NK * 768], wdt, tag="slot",
                                   name=f"slot{rep}_{t}")
                for k in range(NK):
                    if variant == "full":
                        src_ap = cc_out[rep][t - 1][k * 128:(k + 1) * 128, :]
                    else:
                        src_ap = cc_in[rep][t - 1][:]
                    nc.sync.dma_start(
                        slot[:, k * 768:k * 768 + 512], src_ap)
                    nc.vector.tensor_add(
                        slot[:, k * 768 + 512:k * 768 + 768],
                        slot[:, k * 768:k * 768 + 256],
                        slot[:, k * 768 + 256:k * 768 + 512],
                    )
                slots[rep, t] = slot

            # Software-pipelined emission: before each close(t) the PE queue
            # holds xA(t+1) + uA(t) — enough independent work to cover the
            # drain->DMA->AllGather->slot-DMA serial chain of step t-1.
            for rep in range(reps):
                if pipeline:
                    emit_xA(1)
                    emit_epilogue(1)
                    emit_xA(2)
                    for t in range(2, 9):
                        if t < 8:
                            emit_xA(t + 1)
                        emit_uA(t)
                        emit_close(t)
                        emit_epilogue(t)
                else:
                    for t in range(1, 9):
                        emit_xA(t)
                        emit_uA(t)
                        if t >= 2:
                            emit_close(t)
                        emit_epilogue(t)

    nc.compile()
    return nc


def _get_runner():
    if "runner" in _CACHE:
        return _CACHE["runner"]

    import jax
    from jax.sharding import Mesh, PartitionSpec
    from jax.experimental.shard_map import shard_map
    import concourse.mybir as mybir
    from concourse import bass2jax

    nc = _build_program()
    bass2jax.install_neuronx_cc_hook()
    partition_name = nc.partition_id_tensor.name if nc.partition_id_tensor else None
    in_names, out_names, out_avals, zero_outs = [], [], [], []
    for alloc in nc.m.functions[0].allocations:
        if not isinstance(alloc, mybir.MemoryLocationSet):
            continue
        name = alloc.memorylocations[0].name
        if alloc.kind == "ExternalInput":
            if name != partition_name:
                in_names.append(name)
        elif alloc.kind == "ExternalOutput":
            out_names.append(name)
            shape = tuple(alloc.tensor_shape)
            dtype = mybir.dt.np(alloc.dtype)
            out_avals.append(jax.core.ShapedArray(shape, dtype))
            zero_outs.append(np.zeros(shape, dtype))
    n_params = len(in_names)
    n_outs = len(out_avals)
    all_in = in_names + out_names + ([partition_name] if partition_name else [])
    donate = tuple(range(n_params, n_params + n_outs))

    def _body(*args):
        operands = list(args)
        if partition_name is not None:
            operands.append(bass2jax.partition_id_tensor())
        return tuple(
            bass2jax._bass_exec_p.bind(
                *operands,
                out_avals=tuple(out_avals),
                in_names=tuple(all_in),
                out_names=tuple(out_names),
                lowering_input_output_aliases=(),
                sim_require_finite=True,
                sim_require_nnan=True,
                nc=nc,
            )
        )

    devices = jax.devices()[:N_CORES]
    mesh = Mesh(np.asarray(devices), ("core",))
    sharded = jax.jit(
        shard_map(
            _body, mesh=mesh,
            in_specs=(PartitionSpec("core"),) * (n_params + n_outs),
            out_specs=(PartitionSpec("core"),) * n_outs,
            check_rep=False,
        ),
        donate_argnums=donate,
        keep_unused=True,
    )
    runner = {
        "sharded": sharded,
        "in_names": in_names,
        "out_names": out_names,
        "out_avals": out_avals,
        "zero_outs": zero_outs,
        "mesh": mesh,
    }
    _CACHE["runner"] = runner
    return runner


def prepare_inputs(x, A_real, A_imag):
    """Host-side reorder/transpose into the kernel's DMA-friendly layouts."""
    import ml_dtypes
    wnp = np.dtype(ml_dtypes.bfloat16)
    x = np.asarray(x, dtype=np.float32)
    A_real = np.asarray(A_real, dtype=np.float32)
    A_imag = np.asarray(A_imag, dtype=np.float32)
    idx = np.concatenate([[0], np.arange(L - 1, 0, -1)]).astype(np.int64)
    Agr = A_real[idx]          # [p, m, n(k)]
    Agi = A_imag[idx]
    D = Agi - Agr
    N2 = -(Agr + Agi)
    # transposed views [p, k, m]
    AgrT = np.ascontiguousarray(Agr.transpose(0, 2, 1)).astype(wnp)
    DT = np.ascontiguousarray(D.transpose(0, 2, 1)).astype(wnp)
    N2T = np.ascontiguousarray(N2.transpose(0, 2, 1)).astype(wnp)

    def percore(matT, c, p_lo):
        # matT: [p, k(M), m(M)] bf16 -> [128, (p,kt,m)] for rows K_c
        sl = matT[p_lo:, :, c * 128:(c + 1) * 128]    # [P, 1024, 128]
        P = sl.shape[0]
        sl = sl.reshape(P, NK, 128, 128)               # [P, kt, kk, m]
        sl = sl.transpose(2, 0, 1, 3)                  # [kk, P, kt, m]
        return np.ascontiguousarray(sl.reshape(128, P * NK * 128))

    wars = [percore(AgrT, c, 0) for c in range(N_CORES)]
    wds = [percore(DT, c, 0) for c in range(N_CORES)]
    wn2s = [percore(N2T, c, 1) for c in range(N_CORES)]
    # x: [b, q, m] -> [128, (q, kt, b)], identical on every core
    xt = x.transpose(2, 1, 0).astype(wnp)              # [M, q, B]
    xt = xt.reshape(NK, 128, L, B).transpose(1, 2, 0, 3)  # [kk, q, kt, b]
    xw = np.ascontiguousarray(xt.reshape(128, L * NK * B))
    xws = [xw] * N_CORES
    return wars, wds, wn2s, xws


def kernel(x, A_real, A_imag, predict_length):
    P = int(predict_length)
    if P != P_STEPS:  # pragma: no cover - reference always uses 8
        return _numpy_fallback(x, A_real, A_imag, P)

    import jax

    runner = _get_runner()
    wars, wds, wn2s, xws = prepare_inputs(x, A_real, A_imag)
    in_maps = [
        {"war": wars[c], "wd": wds[c], "wn2": wn2s[c], "xw": xws[c]}
        for c in range(N_CORES)
    ]
    concat_in = [
        np.concatenate([m[n] for m in in_maps], axis=0)
        for n in runner["in_names"]
    ]
    czeros = [
        np.zeros((N_CORES * z.shape[0], *z.shape[1:]), z.dtype)
        for z in runner["zero_outs"]
    ]
    out_arrs = runner["sharded"](*concat_in, *czeros)
    jax.block_until_ready(out_arrs)
    o = np.asarray(out_arrs[0]).astype(np.float32)
    o = o.reshape(N_CORES, P_STEPS, 128, 256)
    # o: [c, t, r, b] -> [b, t, c*128+r]
    full = np.ascontiguousarray(
        o.transpose(3, 1, 0, 2).reshape(B, P_STEPS, M))
    return full


def _numpy_fallback(x, A_real, A_imag, P):
    A = (np.asarray(A_real) + 1j * np.asarray(A_imag)).astype(np.complex64)
    idx = np.concatenate([[0], np.arange(L - 1, 0, -1)]).astype(np.int64)
    Ag = A[idx]
    uc = np.asarray(x).astype(np.complex64)
    for _ in range(P):
        u2 = np.einsum("kmn,bkn->bm", Ag, uc)
        uc = np.concatenate([uc[:, 1:], u2[:, None]], axis=1)
    return np.real(uc).astype(np.float32)


# revision 8
# speedup vs baseline: 1.1000x; 1.1000x over previous
"""Trainium2 Bass kernel for the 8-step complex DMD recurrence — v2.

Math (matching the reference):
  Ag[0]=A[0], Ag[p]=A[8-p] (p>=1), all complex [M,M].
  window w_t (len 8) starts as the real inputs x_0..x_7; each step
    u2_t = sum_p Ag[p] @ w_t[p]   (complex, [B,M])
  then the window slides.  Output = Re([u2_1..u2_8]) as [B, 8, M].

Strategy v2 (m-row sharding + per-step AllGather + 3-mult complex):
  * core c owns OUTPUT rows K_c = [128c, 128c+128) of the M dim and holds
    row slices Ag[p][K_c, :] of every operator.  Each step it computes the
    fully-reduced u2_t[K_c] locally (contraction over all of M) — no
    cross-core reduction needed, and each step's output rows stream
    straight to the per-core out tensor.
  * u2_t must then be broadcast for the next step's contraction: a 1-chip
    AllGather of the [128, 512] bf16 [re|im] shard (measured AG floor
    ~4.6us vs ReduceScatter ~7.3us; AG also avoids the CCE double-read).
  * complex products use the 3-multiplication (Gauss) form with
    host-precomputed stationaries War=Ar, Wd=Ai-Ar, Wn2=-(Ar+Ai):
      S  += Ar  @ (ur+ui)      (also re-part of x terms: S += Ar @ x)
      RE += Wn2 @ ui
      IM += Wd  @ ur           (also im-part of x terms: IM += Wd @ x)
      re = RE + S,  im = IM + S   (two DVE adds at drain time)
    -> 24 matmuls per complex window position instead of 32, and x terms
    need only War/Wd (Ai itself never ships).
  * per step the serial chain is just: close matmuls (newest u2 term) ->
    ACT stage + 2 DVE adds -> DMA [128,512] -> AllGather -> slot DMA +
    us add.  The x-terms and older-u2 terms of each step are emitted
    before its close, keeping the PE busy through the AllGather wait
    (deeper lookahead measured slower: it delays the close on the
    in-order PE queue).
  * step 8 computes only re, accumulated in a single PSUM region.
"""

import numpy as np

B, L, M = 256, 8, 1024
N_CORES = 8
NK = M // 128   # 8 contraction tiles
P_STEPS = 8

IN_NAMES = ("war", "wd", "wn2", "xw", "wgr", "wgni")

_CACHE = {}


def _build_program(reps=1, variant="full", pipeline=False):
    import concourse.bacc as bacc
    import concourse.mybir as mybir
    import concourse.tile as tile
    from concourse.bass import ts

    dt = mybir.dt
    f32 = dt.float32
    bf16 = dt.bfloat16
    wdt = bf16   # stationaries + x + slots
    cdt = bf16   # collective payload
    sdt = bf16   # stage / output element type

    nc = bacc.Bacc("TRN2", target_bir_lowering=False, debug=False,
                   num_devices=N_CORES)

    # Per-core inputs, partition-major layouts prepared on the host:
    #   war/wd: [k=128, (p,kt,m)] -> [128, 8*8*128]   row slices, transposed
    #   wn2:    [128, 7*8*128]                        (p>=1 only)
    #   xw:     [128, (q,kt,b)]  -> [128, 8*8*256]    full x, all cores
    war = nc.dram_tensor("war", [128, L * NK * 128], wdt, kind="ExternalInput")
    wd = nc.dram_tensor("wd", [128, L * NK * 128], wdt, kind="ExternalInput")
    wn2 = nc.dram_tensor("wn2", [128, (L - 1) * NK * 128], wdt,
                         kind="ExternalInput")
    xw = nc.dram_tensor("xw", [128, L * NK * 256], wdt, kind="ExternalInput")
    # G_p = Ag[7] @ Ag[p] composites for the step-8 tail lookahead:
    # wgr = Re(G) all p, wgni = -Im(G) for p>=2 (u-positions only).
    wgr = nc.dram_tensor("wgr", [128, L * NK * 128], wdt, kind="ExternalInput")
    wgni = nc.dram_tensor("wgni", [128, (L - 2) * NK * 128], wdt,
                          kind="ExternalInput")
    out = nc.dram_tensor("out", [P_STEPS, 128, 256], sdt,
                         kind="ExternalOutput")

    # Collective buffers (HBM): shard in, gathered u2_t out. One set per
    # rep so benchmark replication adds no false cross-rep dependencies.
    cc_in = [[nc.dram_tensor(f"cc_in{r}_{t}", [128, 512], cdt)
              for t in range(1, 7)] for r in range(reps)]
    cc_out = [[nc.dram_tensor(f"cc_out{r}_{t}", [M, 512], cdt)
               for t in range(1, 7)] for r in range(reps)]

    rg = [list(range(N_CORES))]

    with tile.TileContext(nc) as tc:
        with (
            tc.tile_pool(name="a", bufs=1) as apool,
            tc.tile_pool(name="slot", bufs=6) as slpool,
            tc.tile_pool(name="stg", bufs=2) as stpool,
            tc.tile_pool(name="ps", bufs=4, space="PSUM") as pspool,
        ):
            t_war = apool.tile([128, L * NK * 128], wdt, tag="war")
            t_wd = apool.tile([128, L * NK * 128], wdt, tag="wd")
            t_wn2 = apool.tile([128, (L - 1) * NK * 128], wdt, tag="wn2")
            t_xw = apool.tile([128, L * NK * 256], wdt, tag="xw")
            t_wgr = apool.tile([128, L * NK * 128], wdt, tag="wgr")
            t_wgni = apool.tile([128, (L - 2) * NK * 128], wdt, tag="wgni")

            def ar(p, k):
                return t_war[:, ts(p * NK + k, 128)]

            def wd_(p, k):
                return t_wd[:, ts(p * NK + k, 128)]

            def n2(p, k):
                return t_wn2[:, ts((p - 1) * NK + k, 128)]

            def xv(q, k):
                return t_xw[:, ts(q * NK + k, 256)]

            def gr(p, k):
                return t_wgr[:, ts(p * NK + k, 128)]

            def gni(p, k):
                return t_wgni[:, ts((p - 2) * NK + k, 128)]

            # Head loads: interleave x / war / wd per position so position-0
            # matmuls start after ~1MB instead of 8MB.  wn2 (first consumed
            # by step 2's close, which always uses position 7) queues after
            # the step-1-critical loads, position 7 first.
            for p in range(L):
                nc.sync.dma_start(t_xw[:, ts(p, NK * 256)],
                                  xw[:, ts(p, NK * 256)])
                nc.sync.dma_start(t_war[:, ts(p, NK * 128)],
                                  war[:, ts(p, NK * 128)])
                nc.sync.dma_start(t_wd[:, ts(p, NK * 128)],
                                  wd[:, ts(p, NK * 128)])
            for p in (7, 1, 2, 3, 4, 5, 6):
                nc.sync.dma_start(t_wn2[:, ts(p - 1, NK * 128)],
                                  wn2[:, ts(p - 1, NK * 128)])
            # G composites are only consumed by step 8 — load last.
            nc.sync.dma_start(t_wgr[:], wgr[:])
            nc.sync.dma_start(t_wgni[:], wgni[:])

            slots = {}  # (rep, j) -> sbuf tile [128, 8*768] per-ktile [ur|ui|us]
            if variant == "nocc":
                dummy = slpool.tile([128, NK * 768], wdt, tag="dummy")
                nc.vector.memset(dummy[:], 0.0)
                slots["dummy"] = dummy

            rep = 0

            def slot_ap(j, k, part):
                lo = k * 768 + part * 256
                return slots[rep, j][:, lo:lo + 256]

            # PSUM region bases within a [128, 1024] (= 2 bank) step tile.
            # PSUM group start/stop is BANK-granular (start zeroes the whole
            # 2KB bank), so S|RE share bank 0 with ONE start, IM is bank 1.
            S_, RE_, IM_ = 0, 256, 512

            pss = {}      # (rep, t) -> psum tile
            started = {}  # (rep, t) -> [bank0, bank1]

            def mm(t, region, lhsT, rhs, stop=False):
                st = started[rep, t]
                bank = 0 if region < 512 else 1
                nc.tensor.matmul(
                    pss[rep, t][:, region:region + 256], lhsT, rhs,
                    start=not st[bank], stop=stop,
                    skip_group_check=True,
                )
                st[bank] = True

            def emit_xA(t):
                """x-only terms of step t — no u2 dependencies at all."""
                pss[rep, t] = pspool.tile([128, 1024], f32, tag="ps",
                                          name=f"ps{rep}_{t}")
                started[rep, t] = [False, False]
                if t < 7:
                    for p in range(0, 9 - t):
                        q = p + t - 1
                        # t=1 has no close phase: its x groups carry the stops
                        cl = t == 1 and p == 8 - t
                        for k in range(NK):
                            st = cl and k == NK - 1
                            mm(t, S_, ar(p, k), xv(q, k), stop=st)
                            mm(t, IM_, wd_(p, k), xv(q, k), stop=st)
                else:
                    # steps 7 and 8 are re-only (u2_7's im is never consumed
                    # once step 8 closes via the G composites)
                    for p in range(0, 9 - t):
                        for k in range(NK):
                            mm(t, RE_, ar(p, k), xv(p + t - 1, k))

            def emit_uA(t):
                """older-u2 terms of step t (slots j <= t-2)."""
                hi = t - 1 if t < 8 else 7  # step 8 has no slot_7: all j<=6
                for j in range(1, hi):
                    pos = 8 - t + j
                    for k in range(NK):
                        if t < 7:
                            mm(t, S_, ar(pos, k), slot_ap(j, k, 2))
                            mm(t, RE_, n2(pos, k), slot_ap(j, k, 1))
                            mm(t, IM_, wd_(pos, k), slot_ap(j, k, 0))
                        else:
                            mm(t, RE_, ar(pos, k), slot_ap(j, k, 2))
                            mm(t, RE_, n2(pos, k), slot_ap(j, k, 1))

            def emit_close(t):
                """newest term j = t-1, position 7 — waits on slot t-1.

                Region order S -> RE -> IM so S (and then RE) close early:
                the ACT stage-copy of S and the re-combine DVE add overlap
                the remaining RE/IM close matmuls."""
                j = t - 1
                if t < 7:
                    for k in range(NK):
                        mm(t, S_, ar(7, k), slot_ap(j, k, 2),
                           stop=k == NK - 1)
                    for k in range(NK):
                        mm(t, RE_, n2(7, k), slot_ap(j, k, 1),
                           stop=k == NK - 1)
                    for k in range(NK):
                        mm(t, IM_, wd_(7, k), slot_ap(j, k, 0),
                           stop=k == NK - 1)
                elif t == 7:
                    for k in range(NK):
                        mm(t, RE_, n2(7, k), slot_ap(j, k, 1))
                        mm(t, RE_, ar(7, k), slot_ap(j, k, 2),
                           stop=k == NK - 1)
                else:
                    # step 8 G-form: Ag[7] @ u2_7 expanded over x and
                    # slots 1..6 via G_p = Ag[7] @ Ag[p] — no AG_7 needed.
                    for p in (0, 1):
                        for k in range(NK):
                            mm(t, RE_, gr(p, k), xv(p + 6, k))
                    for jj in range(1, 7):
                        for k in range(NK):
                            mm(t, RE_, gr(jj + 1, k), slot_ap(jj, k, 0))
                            mm(t, RE_, gni(jj + 1, k), slot_ap(jj, k, 1),
                               stop=jj == 6 and k == NK - 1)

            def emit_epilogue(t):
                last = t >= 7
                ps = pss[rep, t]
                if last:
                    stg = stpool.tile([128, 256], sdt, tag="stg8",
                                      name=f"stg8_{rep}_{t}")
                    nc.vector.tensor_copy(stg[:], ps[:, RE_:RE_ + 256])
                    nc.sync.dma_start(out[t - 1], stg[:])
                    return
                # DVE tensor_tensor cannot read two PSUM operands (neuronxcc
                # verifier); stage S to SBUF f32 on the ACT engine first.
                stg = stpool.tile([128, 512], sdt, tag="stg", name=f"stg{rep}_{t}")
                s_f32 = stpool.tile([128, 256], f32, tag="sf32",
                                    name=f"sf32_{rep}_{t}")
                nc.scalar.copy(s_f32[:], ps[:, S_:S_ + 256])
                if t == 1:
                    nc.vector.tensor_copy(stg[:, 0:256], ps[:, S_:S_ + 256])
                else:
                    nc.vector.tensor_add(stg[:, 0:256], ps[:, RE_:RE_ + 256],
                                         s_f32[:])
                nc.vector.tensor_add(stg[:, 256:512], ps[:, IM_:IM_ + 256],
                                     s_f32[:])
                # out DMA rides the ACT engine's HWDGE ring so it never
                # queues ahead of the chain-critical cc_in DMA on SP's ring.
                nc.scalar.dma_start(out[t - 1], stg[:, 0:256])
                if variant == "nocc":
                    slots[rep, t] = slots["dummy"]
                    return
                nc.sync.dma_start(cc_in[rep][t - 1][:], stg[:])
                if variant == "full":
                    nc.gpsimd.collective_compute(
                        "AllGather", mybir.AluOpType.bypass,
                        replica_groups=rg,
                        ins=[cc_in[rep][t - 1][:]],
                        outs=[cc_out[rep][t - 1][:]],
                    )
                slot = slpool.tile([128, NK * 768], wdt, tag="slot",
                                   name=f"slot{rep}_{t}")
                for k in range(NK):
                    if variant == "full":
                        src_ap = cc_out[rep][t - 1][k * 128:(k + 1) * 128, :]
                    else:
                        src_ap = cc_in[rep][t - 1][:]
                    nc.sync.dma_start(
                        slot[:, k * 768:k * 768 + 512], src_ap)
                    nc.vector.tensor_add(
                        slot[:, k * 768 + 512:k * 768 + 768],
                        slot[:, k * 768:k * 768 + 256],
                        slot[:, k * 768 + 256:k * 768 + 512],
                    )
                slots[rep, t] = slot

            # Software-pipelined emission: before each close(t) the PE queue
            # holds xA(t+1) + uA(t) — enough independent work to cover the
            # drain->DMA->AllGather->slot-DMA serial chain of step t-1.
            for rep in range(reps):
                if pipeline:
                    emit_xA(1)
                    emit_epilogue(1)
                    emit_xA(2)
                    for t in range(2, 9):
                        if t < 8:
                            emit_xA(t + 1)
                        emit_uA(t)
                        emit_close(t)
                        emit_epilogue(t)
                else:
                    for t in range(1, 9):
                        emit_xA(t)
                        emit_uA(t)
                        if t >= 2:
                            emit_close(t)
                        emit_epilogue(t)

    nc.compile()
    return nc


def _get_runner():
    if "runner" in _CACHE:
        return _CACHE["runner"]

    import jax
    from jax.sharding import Mesh, PartitionSpec
    from jax.experimental.shard_map import shard_map
    import concourse.mybir as mybir
    from concourse import bass2jax

    nc = _build_program()
    bass2jax.install_neuronx_cc_hook()
    partition_name = nc.partition_id_tensor.name if nc.partition_id_tensor else None
    in_names, out_names, out_avals, zero_outs = [], [], [], []
    for alloc in nc.m.functions[0].allocations:
        if not isinstance(alloc, mybir.MemoryLocationSet):
            continue
        name = alloc.memorylocations[0].name
        if alloc.kind == "ExternalInput":
            if name != partition_name:
                in_names.append(name)
        elif alloc.kind == "ExternalOutput":
            out_names.append(name)
            shape = tuple(alloc.tensor_shape)
            dtype = mybir.dt.np(alloc.dtype)
            out_avals.append(jax.core.ShapedArray(shape, dtype))
            zero_outs.append(np.zeros(shape, dtype))
    n_params = len(in_names)
    n_outs = len(out_avals)
    all_in = in_names + out_names + ([partition_name] if partition_name else [])
    donate = tuple(range(n_params, n_params + n_outs))

    def _body(*args):
        operands = list(args)
        if partition_name is not None:
            operands.append(bass2jax.partition_id_tensor())
        return tuple(
            bass2jax._bass_exec_p.bind(
                *operands,
                out_avals=tuple(out_avals),
                in_names=tuple(all_in),
                out_names=tuple(out_names),
                lowering_input_output_aliases=(),
                sim_require_finite=True,
                sim_require_nnan=True,
                nc=nc,
            )
        )

    devices = jax.devices()[:N_CORES]
    mesh = Mesh(np.asarray(devices), ("core",))
    sharded = jax.jit(
        shard_map(
            _body, mesh=mesh,
            in_specs=(PartitionSpec("core"),) * (n_params + n_outs),
            out_specs=(PartitionSpec("core"),) * n_outs,
            check_rep=False,
        ),
        donate_argnums=donate,
        keep_unused=True,
    )
    runner = {
        "sharded": sharded,
        "in_names": in_names,
        "out_names": out_names,
        "out_avals": out_avals,
        "zero_outs": zero_outs,
        "mesh": mesh,
    }
    _CACHE["runner"] = runner
    return runner


def prepare_inputs(x, A_real, A_imag):
    """Host-side reorder/transpose into the kernel's DMA-friendly layouts."""
    import ml_dtypes
    wnp = np.dtype(ml_dtypes.bfloat16)
    x = np.asarray(x, dtype=np.float32)
    A_real = np.asarray(A_real, dtype=np.float32)
    A_imag = np.asarray(A_imag, dtype=np.float32)
    idx = np.concatenate([[0], np.arange(L - 1, 0, -1)]).astype(np.int64)
    Agr = A_real[idx]          # [p, m, n(k)]
    Agi = A_imag[idx]
    D = Agi - Agr
    N2 = -(Agr + Agi)
    # transposed views [p, k, m]
    AgrT = np.ascontiguousarray(Agr.transpose(0, 2, 1)).astype(wnp)
    DT = np.ascontiguousarray(D.transpose(0, 2, 1)).astype(wnp)
    N2T = np.ascontiguousarray(N2.transpose(0, 2, 1)).astype(wnp)

    def percore(matT, c, p_lo):
        # matT: [p, k(M), m(M)] bf16 -> [128, (p,kt,m)] for rows K_c
        sl = matT[p_lo:, :, c * 128:(c + 1) * 128]    # [P, 1024, 128]
        P = sl.shape[0]
        sl = sl.reshape(P, NK, 128, 128)               # [P, kt, kk, m]
        sl = sl.transpose(2, 0, 1, 3)                  # [kk, P, kt, m]
        return np.ascontiguousarray(sl.reshape(128, P * NK * 128))

    wars = [percore(AgrT, c, 0) for c in range(N_CORES)]
    wds = [percore(DT, c, 0) for c in range(N_CORES)]
    wn2s = [percore(N2T, c, 1) for c in range(N_CORES)]
    # G_p = Ag[7] @ Ag[p] composites (complex64 BLAS) for the step-8 tail
    Agc = (Agr + 1j * Agi).astype(np.complex64)
    G = np.matmul(Agc[7][None], Agc)            # [p, m, k]
    GrT = np.ascontiguousarray(np.real(G).transpose(0, 2, 1)).astype(wnp)
    GniT = np.ascontiguousarray((-np.imag(G)).transpose(0, 2, 1)).astype(wnp)
    wgrs = [percore(GrT, c, 0) for c in range(N_CORES)]
    wgnis = [percore(GniT, c, 2) for c in range(N_CORES)]
    # x: [b, q, m] -> [128, (q, kt, b)], identical on every core
    xt = x.transpose(2, 1, 0).astype(wnp)              # [M, q, B]
    xt = xt.reshape(NK, 128, L, B).transpose(1, 2, 0, 3)  # [kk, q, kt, b]
    xw = np.ascontiguousarray(xt.reshape(128, L * NK * B))
    xws = [xw] * N_CORES
    return wars, wds, wn2s, xws, wgrs, wgnis


def kernel(x, A_real, A_imag, predict_length):
    P = int(predict_length)
    if P != P_STEPS:  # pragma: no cover - reference always uses 8
        return _numpy_fallback(x, A_real, A_imag, P)

    import jax

    runner = _get_runner()
    prep = prepare_inputs(x, A_real, A_imag)
    in_maps = [
        {n: arrs[c] for n, arrs in zip(IN_NAMES, prep)}
        for c in range(N_CORES)
    ]
    concat_in = [
        np.concatenate([m[n] for m in in_maps], axis=0)
        for n in runner["in_names"]
    ]
    czeros = [
        np.zeros((N_CORES * z.shape[0], *z.shape[1:]), z.dtype)
        for z in runner["zero_outs"]
    ]
    out_arrs = runner["sharded"](*concat_in, *czeros)
    jax.block_until_ready(out_arrs)
    o = np.asarray(out_arrs[0]).astype(np.float32)
    o = o.reshape(N_CORES, P_STEPS, 128, 256)
    # o: [c, t, r, b] -> [b, t, c*128+r]
    full = np.ascontiguousarray(
        o.transpose(3, 1, 0, 2).reshape(B, P_STEPS, M))
    return full


def _numpy_fallback(x, A_real, A_imag, P):
    A = (np.asarray(A_real) + 1j * np.asarray(A_imag)).astype(np.complex64)
    idx = np.concatenate([[0], np.arange(L - 1, 0, -1)]).astype(np.int64)
    Ag = A[idx]
    uc = np.asarray(x).astype(np.complex64)
    for _ in range(P):
        u2 = np.einsum("kmn,bkn->bm", Ag, uc)
        uc = np.concatenate([uc[:, 1:], u2[:, None]], axis=1)
    return np.real(uc).astype(np.float32)
